# revision 24
# baseline (speedup 1.0000x reference)
"""FP8-weight dense linear (FFN up-proj) on 8 Trainium2 NeuronCores.

Computes out[128, 16384] = x[128, 4096] @ dequant(weight_fp8[16384, 4096]).T
+ bias, tensor-parallel: weight/bias sharded along out_features (2048 rows
per core), x replicated, output gathered by concatenation (no collectives).

Per-core kernel design:
- The PE contracts over the partition dim, so the fp8 weight needs
  in_features on partitions. The HW xbar DMA-transpose only supports 2-byte
  elements, so the weight shard is viewed as uint16 (adjacent fp8 pairs
  along in_features) and streamed through the xbar in chunks. Each
  transposed SBUF tile holds fp8 pairs (i=2j, 2j+1) interleaved along the
  free dim; the matmul reads them as stride-2 fp8 APs (even/odd parity),
  paired with a matching even/odd-deinterleaved, transposed fp16 copy of x
  as the stationary operand.
- Mixed-dtype matmul (fp16 stationary x, fp8 moving weights) dequantizes
  in the PE itself at full rate; accumulation in fp32 PSUM.
- Bias enters as a rank-1 fp32 matmul (ones[1,128].T @ bias[1,512]) that
  opens each PSUM accumulation group.
"""

import sys

if "/opt/trn_rl_repo" not in sys.path:
    sys.path.insert(0, "/opt/trn_rl_repo")

import os

import numpy as np

import concourse.bass as bass  # noqa: F401  (registers bass lowering)
import concourse.mybir as mybir
import concourse.tile as tile
from concourse import bacc
from concourse.bass_utils import run_bass_kernel_spmd

if os.environ.get("LDW_OPT", "0") == "1":
    # The quad-MM loop issues 4 consecutive matmuls sharing one stationary
    # operand; walrus can elide the 3 redundant LDWEIGHTS but ships with
    # --enable-ldw-opt=false. Rewrite the flag at walrus invocation.
    import concourse.bass_utils as _bu

    _orig_run_command = _bu.run_command

    def _run_command_ldw(argv, **kwargs):
        argv = ["--enable-ldw-opt=true" if a == "--enable-ldw-opt=false"
                else a for a in argv]
        return _orig_run_command(argv, **kwargs)

    _bu.run_command = _run_command_ldw

N_CORES = 8
T = 128          # tokens
K = 4096         # in_features
O_FULL = 16384   # out_features
O = O_FULL // N_CORES  # 2048 per core
O_CHUNK = 512    # psum bank / matmul free dim
N_OSL = O // O_CHUNK   # 4 o-slices per core
JT = K // 2 // 128     # 16 pair-tiles (each covers 256 in_features)

_NC = None


def _build_nc(repeats: int = 1, xpath: str = "pe", out_split: bool = False,
              evict: str = "vector", chunked: bool = False,
              mm_order: str = "osl_inner", wbufs: int = 3):
    """Build the per-core BIR. repeats>1 wraps the whole computation in a
    hardware For_i loop — used only for benchmarking (wall-clock differencing
    across repeat counts cancels the ~80ms axon dispatch overhead).

    xpath: "xbar" = x deinterleave/transpose via DMA xbar;
           "pe"   = x cast-loaded via SWDGE, transposed on the PE (keeps the
                    xbar stream pure weight transposes, no mode switches).
    """
    from concourse.masks import make_identity
    nc = bacc.Bacc("TRN2", target_bir_lowering=False, debug=False,
                   num_devices=N_CORES)
    x_d = nc.dram_tensor("x", [T, K], mybir.dt.float32, kind="ExternalInput")
    w_d = nc.dram_tensor("w16", [O, K // 2], mybir.dt.uint16,
                         kind="ExternalInput")
    b_d = nc.dram_tensor("bias", [1, O], mybir.dt.float32,
                         kind="ExternalInput")
    o_d = nc.dram_tensor("out", [T, O], mybir.dt.float32,
                         kind="ExternalOutput")

    with tile.TileContext(nc) as tc:
        with (
            tc.tile_pool(name="const", bufs=1) as const,
            tc.tile_pool(name="xpool", bufs=1) as xpool,
            tc.tile_pool(name="wpool", bufs=wbufs) as wpool,
            tc.tile_pool(name="opool", bufs=1) as opool,
            tc.tile_pool(name="psum", bufs=4, space="PSUM") as psum,
        ):
            identity = None
            if xpath == "pe":
                identity = const.tile([128, 128], mybir.dt.float16,
                                      name="identity")
                make_identity(nc, identity)

            def body():
                ones = const.tile([1, T], mybir.dt.float32)
                nc.any.memset(ones[:], 1.0)
                bias_sb = const.tile([1, O], mybir.dt.float32)
                nc.sync.dma_start(bias_sb[:], b_d.ap())

                def w_dma(osl):
                    # wt[p, jt, o] = w16[osl*512 + o, jt*128 + p]
                    # (contiguous HBM read: a row-slice of the shard)
                    wt = wpool.tile([128, JT, O_CHUNK], mybir.dt.uint16,
                                    name=f"wt{osl}", tag="wt")
                    nc.sync.dma_start(
                        wt[:], w_d.ap()[osl * O_CHUNK:(osl + 1) * O_CHUNK, :],
                        transpose=True)
                    return wt

                xTe = xpool.tile([128, JT, T], mybir.dt.float16)
                xTo = xpool.tile([128, JT, T], mybir.dt.float16)

                if xpath == "xbar":
                    x32 = xpool.tile([T, K], mybir.dt.float32)
                    nc.sync.dma_start(x32[:], x_d.ap())
                    # xbar order: w0 | xeT xoT | w1 w2 w3 — the x transposes
                    # land when their casts finish, w stream fills the rest
                    wts = [w_dma(0)]
                    # deinterleave + cast: xe[t, j] = fp16(x[t, 2j]), xo odd
                    # (one on DVE, one on ACT so they run concurrently)
                    xe = xpool.tile([T, K // 2], mybir.dt.float16)
                    xo = xpool.tile([T, K // 2], mybir.dt.float16)
                    nc.vector.tensor_copy(xe[:], x32[:, 0::2])
                    nc.scalar.copy(xo[:], x32[:, 1::2])
                    # transpose to [pi, jt, t]: xT?[p, jt, t] = x?[t, jt*128+p]
                    nc.sync.dma_start(xTe[:], xe[:], transpose=True)
                    nc.sync.dma_start(xTo[:], xo[:], transpose=True)
                    for osl in range(1, N_OSL):
                        wts.append(w_dma(osl))
                else:
                    # x via SWDGE cast-load + PE transposes (xbar untouched)
                    x16 = xpool.tile([T, K], mybir.dt.float16)
                    nc.gpsimd.dma_start(x16[:], x_d.ap())
                    wts = [w_dma(osl) for osl in range(N_OSL)]
                    for jt in range(JT):
                        for par, xT in ((0, xTe), (1, xTo)):
                            tp = psum.tile([128, T], mybir.dt.float16,
                                           name=f"tp{jt}_{par}", tag="tpsum")
                            nc.tensor.transpose(
                                tp[:], x16[:, 256 * jt + par::2][:, :128],
                                identity[:])
                            nc.vector.tensor_copy(xT[:, jt, :], tp[:])

                out_sb = opool.tile([T, O], mybir.dt.float32)
                ev = (nc.scalar.copy if evict == "scalar"
                      else nc.vector.tensor_copy)

                for osl in range(N_OSL):
                    wt8 = wts[osl][:].bitcast(mybir.dt.float8e4)
                    ps = psum.tile([T, O_CHUNK], mybir.dt.float32,
                                   name=f"ps{osl}", tag="ps")
                    nc.tensor.matmul(
                        ps[:], ones[:],
                        bias_sb[:, osl * O_CHUNK:(osl + 1) * O_CHUNK],
                        start=True, stop=False)
                    for jt in range(JT):
                        for par, xT in ((0, xTe), (1, xTo)):
                            nc.tensor.matmul(
                                ps[:], xT[:, jt, :], wt8[:, jt, par::2],
                                start=False, stop=(jt == JT - 1 and par == 1))
                    ev(out_sb[:, osl * O_CHUNK:(osl + 1) * O_CHUNK], ps[:])
                    if out_split:
                        nc.gpsimd.dma_start(
                            o_d.ap()[:, osl * O_CHUNK:(osl + 1) * O_CHUNK],
                            out_sb[:, osl * O_CHUNK:(osl + 1) * O_CHUNK])

                if not out_split:
                    if xpath == "pe":
                        nc.gpsimd.dma_start(o_d.ap(), out_sb[:])
                    else:
                        nc.sync.dma_start(o_d.ap(), out_sb[:])

            def body_chunked():
                """jt-chunked w stream: 8 xbar transposes of [128, 2, 2048]
                (1 MB each), each feeding 16 MMs spread over 4 live psum
                banks; all 4 accumulation groups stay open across chunks."""
                N_KT = 8
                JTL = JT // N_KT  # jt per chunk
                ones = const.tile([1, T], mybir.dt.float32)
                nc.any.memset(ones[:], 1.0)
                bias_sb = const.tile([1, O], mybir.dt.float32)
                nc.sync.dma_start(bias_sb[:], b_d.ap())

                x16 = xpool.tile([T, K], mybir.dt.float16)
                nc.gpsimd.dma_start(x16[:], x_d.ap())

                xTe = xpool.tile([128, JT, T], mybir.dt.float16)
                xTo = xpool.tile([128, JT, T], mybir.dt.float16)
                for jt in range(JT):
                    for par, xT in ((0, xTe), (1, xTo)):
                        tp = psum.tile([128, T], mybir.dt.float16,
                                       name=f"tp{jt}_{par}", tag="tpsum")
                        nc.tensor.transpose(
                            tp[:], x16[:, 256 * jt + par::2][:, :128],
                            identity[:])
                        nc.vector.tensor_copy(xT[:, jt, :], tp[:])

                out_sb = opool.tile([T, O], mybir.dt.float32)
                ev = (nc.scalar.copy if evict == "scalar"
                      else nc.vector.tensor_copy)

                pss = []
                for osl in range(N_OSL):
                    ps = psum.tile([T, O_CHUNK], mybir.dt.float32,
                                   name=f"ps{osl}", tag="ps")
                    nc.tensor.matmul(
                        ps[:], ones[:],
                        bias_sb[:, osl * O_CHUNK:(osl + 1) * O_CHUNK],
                        start=True, stop=False)
                    pss.append(ps)

                for kt in range(N_KT - 1):
                    wt = wpool.tile([128, JTL, K // 2], mybir.dt.uint16,
                                    name=f"wt{kt}", tag="wt")
                    nc.sync.dma_start(
                        wt[:],
                        w_d.ap()[:, kt * JTL * 128:(kt + 1) * JTL * 128],
                        transpose=True)
                    wt8 = wt[:].bitcast(mybir.dt.float8e4)  # [128, JTL, K]
                    if mm_order == "osl_inner":
                        for jtl in range(JTL):
                            jt = kt * JTL + jtl
                            for par, xT in ((0, xTe), (1, xTo)):
                                for osl in range(N_OSL):
                                    rhs = wt8[:, jtl, par::2][
                                        :, osl * O_CHUNK:(osl + 1) * O_CHUNK]
                                    nc.tensor.matmul(
                                        pss[osl][:], xT[:, jt, :], rhs,
                                        start=False, stop=False)
                    else:
                        for osl in range(N_OSL):
                            for jtl in range(JTL):
                                jt = kt * JTL + jtl
                                for par, xT in ((0, xTe), (1, xTo)):
                                    rhs = wt8[:, jtl, par::2][
                                        :, osl * O_CHUNK:(osl + 1) * O_CHUNK]
                                    nc.tensor.matmul(
                                        pss[osl][:], xT[:, jt, :], rhs,
                                        start=False, stop=False)

                # last chunk: osl-outer so each psum group closes early;
                # evict + store overlap the remaining groups' matmuls
                kt = N_KT - 1
                wt = wpool.tile([128, JTL, K // 2], mybir.dt.uint16,
                                name=f"wt{kt}", tag="wt")
                nc.sync.dma_start(
                    wt[:], w_d.ap()[:, kt * JTL * 128:(kt + 1) * JTL * 128],
                    transpose=True)
                wt8 = wt[:].bitcast(mybir.dt.float8e4)
                for osl in range(N_OSL):
                    for jtl in range(JTL):
                        jt = kt * JTL + jtl
                        for par, xT in ((0, xTe), (1, xTo)):
                            rhs = wt8[:, jtl, par::2][
                                :, osl * O_CHUNK:(osl + 1) * O_CHUNK]
                            nc.tensor.matmul(
                                pss[osl][:], xT[:, jt, :], rhs, start=False,
                                stop=(jtl == JTL - 1 and par == 1))
                    ev(out_sb[:, osl * O_CHUNK:(osl + 1) * O_CHUNK],
                       pss[osl][:])
                    nc.gpsimd.dma_start(
                        o_d.ap()[:, osl * O_CHUNK:(osl + 1) * O_CHUNK],
                        out_sb[:, osl * O_CHUNK:(osl + 1) * O_CHUNK])

            def body_half():
                """8 xbar transposes of 1 MB o-half-slices [512 o, 1024 j]
                (2 KB contiguous rows), osl-major so each psum group closes
                after its second chunk; evict+store overlap later chunks."""
                ones = const.tile([1, T], mybir.dt.float32)
                nc.any.memset(ones[:], 1.0)
                bias_sb = const.tile([1, O], mybir.dt.float32)
                nc.sync.dma_start(bias_sb[:], b_d.ap())

                x16 = xpool.tile([T, K], mybir.dt.float16)
                nc.gpsimd.dma_start(x16[:], x_d.ap())

                xTe = xpool.tile([128, JT, T], mybir.dt.float16)
                xTo = xpool.tile([128, JT, T], mybir.dt.float16)
                for jt in range(JT):
                    for par, xT in ((0, xTe), (1, xTo)):
                        tp = psum.tile([128, T], mybir.dt.float16,
                                       name=f"tp{jt}_{par}", tag="tpsum")
                        nc.tensor.transpose(
                            tp[:], x16[:, 256 * jt + par::2][:, :128],
                            identity[:])
                        nc.vector.tensor_copy(xT[:, jt, :], tp[:])

                out_sb = opool.tile([T, O], mybir.dt.float32)
                ev = (nc.scalar.copy if evict == "scalar"
                      else nc.vector.tensor_copy)

                pss = []
                for osl in range(N_OSL):
                    ps = psum.tile([T, O_CHUNK], mybir.dt.float32,
                                   name=f"ps{osl}", tag="ps")
                    nc.tensor.matmul(
                        ps[:], ones[:],
                        bias_sb[:, osl * O_CHUNK:(osl + 1) * O_CHUNK],
                        start=True, stop=False)
                    pss.append(ps)

                JH = JT // 2  # jt per half-chunk
                for c in range(2 * N_OSL):
                    osl, half = c // 2, c % 2
                    wt = wpool.tile([128, JH, O_CHUNK], mybir.dt.uint16,
                                    name=f"wh{c}", tag="wt")
                    nc.sync.dma_start(
                        wt[:],
                        w_d.ap()[osl * O_CHUNK:(osl + 1) * O_CHUNK,
                                 half * JH * 128:(half + 1) * JH * 128],
                        transpose=True)
                    wt8 = wt[:].bitcast(mybir.dt.float8e4)  # [128, JH, 1024]
                    for jh in range(JH):
                        jt = half * JH + jh
                        for par, xT in ((0, xTe), (1, xTo)):
                            nc.tensor.matmul(
                                pss[osl][:], xT[:, jt, :], wt8[:, jh, par::2],
                                start=False,
                                stop=(half == 1 and jh == JH - 1 and par == 1))
                    if half == 1:
                        ev(out_sb[:, osl * O_CHUNK:(osl + 1) * O_CHUNK],
                           pss[osl][:])
                        nc.gpsimd.dma_start(
                            o_d.ap()[:, osl * O_CHUNK:(osl + 1) * O_CHUNK],
                            out_sb[:, osl * O_CHUNK:(osl + 1) * O_CHUNK])

            def body_final():
                """Fast 2MB o-slice transposes (4KB rows) feeding per-slice
                32-MM bursts into 4 early-opened psum groups; the last slice
                is split into jt-halves so the tail burst is only 16 MMs;
                evict+store per group as it closes."""
                ones = const.tile([1, T], mybir.dt.float32)
                nc.any.memset(ones[:], 1.0)
                bias_sb = const.tile([1, O], mybir.dt.float32)
                nc.sync.dma_start(bias_sb[:], b_d.ap())

                x16 = xpool.tile([T, K], mybir.dt.float16)
                nc.gpsimd.dma_start(x16[:], x_d.ap())

                xTe = xpool.tile([128, JT, T], mybir.dt.float16)
                xTo = xpool.tile([128, JT, T], mybir.dt.float16)
                for jt in range(JT):
                    for par, xT in ((0, xTe), (1, xTo)):
                        tp = psum.tile([128, T], mybir.dt.float16,
                                       name=f"tp{jt}_{par}", tag="tpsum")
                        nc.tensor.transpose(
                            tp[:], x16[:, 256 * jt + par::2][:, :128],
                            identity[:])
                        nc.vector.tensor_copy(xT[:, jt, :], tp[:])

                out_sb = opool.tile([T, O], mybir.dt.float32)
                ev = (nc.scalar.copy if evict == "scalar"
                      else nc.vector.tensor_copy)

                pss = []
                for osl in range(N_OSL):
                    ps = psum.tile([T, O_CHUNK], mybir.dt.float32,
                                   name=f"ps{osl}", tag="ps")
                    nc.tensor.matmul(
                        ps[:], ones[:],
                        bias_sb[:, osl * O_CHUNK:(osl + 1) * O_CHUNK],
                        start=True, stop=False)
                    pss.append(ps)

                def finish(osl):
                    ev(out_sb[:, osl * O_CHUNK:(osl + 1) * O_CHUNK],
                       pss[osl][:])
                    nc.gpsimd.dma_start(
                        o_d.ap()[:, osl * O_CHUNK:(osl + 1) * O_CHUNK],
                        out_sb[:, osl * O_CHUNK:(osl + 1) * O_CHUNK])

                for osl in range(N_OSL - 1):
                    wt = wpool.tile([128, JT, O_CHUNK], mybir.dt.uint16,
                                    name=f"wt{osl}", tag="wt")
                    nc.sync.dma_start(
                        wt[:], w_d.ap()[osl * O_CHUNK:(osl + 1) * O_CHUNK, :],
                        transpose=True)
                    wt8 = wt[:].bitcast(mybir.dt.float8e4)
                    for jt in range(JT):
                        for par, xT in ((0, xTe), (1, xTo)):
                            nc.tensor.matmul(
                                pss[osl][:], xT[:, jt, :], wt8[:, jt, par::2],
                                start=False,
                                stop=(jt == JT - 1 and par == 1))
                    finish(osl)

                # last o-slice in two jt-halves: the tail after the final
                # (slower, 1MB) transpose is only 16 matmuls
                osl = N_OSL - 1
                JH = JT // 2
                for half in range(2):
                    wt = wpool.tile([128, JH, O_CHUNK], mybir.dt.uint16,
                                    name=f"wl{half}", tag="wl")
                    nc.sync.dma_start(
                        wt[:],
                        w_d.ap()[osl * O_CHUNK:(osl + 1) * O_CHUNK,
                                 half * JH * 128:(half + 1) * JH * 128],
                        transpose=True)
                    wt8 = wt[:].bitcast(mybir.dt.float8e4)
                    for jh in range(JH):
                        jt = half * JH + jh
                        for par, xT in ((0, xTe), (1, xTo)):
                            nc.tensor.matmul(
                                pss[osl][:], xT[:, jt, :], wt8[:, jh, par::2],
                                start=False,
                                stop=(half == 1 and jh == JH - 1 and par == 1))
                finish(osl)

            if chunked == "half":
                the_body = body_half
            elif chunked == "final":
                the_body = body_final
            elif chunked:
                the_body = body_chunked
            else:
                the_body = body
            if repeats == 1:
                the_body()
            else:
                with tc.For_i(0, repeats, 1):
                    the_body()

    nc.compile()
    return nc


BEST_CONFIG = dict(xpath="pe", evict="scalar", chunked=True,
                   mm_order="osl_outer")


def _get_nc():
    global _NC
    if _NC is None:
        _NC = _build_nc(**BEST_CONFIG)
    return _NC


def kernel(x, weight_fp8, bias):
    x = np.ascontiguousarray(np.asarray(x), dtype=np.float32)
    w = np.ascontiguousarray(np.asarray(weight_fp8))
    b = np.ascontiguousarray(np.asarray(bias), dtype=np.float32)
    assert x.shape == (T, K) and w.shape == (O_FULL, K)

    nc = _get_nc()
    in_maps = []
    for c in range(N_CORES):
        w_sh = np.ascontiguousarray(w[c * O:(c + 1) * O, :])
        in_maps.append({
            "x": x,
            "w16": w_sh.view(np.uint16),
            "bias": b[c * O:(c + 1) * O].reshape(1, O),
        })
    res = run_bass_kernel_spmd(nc, in_maps, core_ids=list(range(N_CORES)))
    return np.concatenate([res.results[c]["out"] for c in range(N_CORES)],
                          axis=1)
